# revision 24
# baseline (speedup 1.0000x reference)
"""Trainium2 Bass kernel for GAT_MS_2_3 (iterative mean-shift sparse attention).

Self-contained: builds + compiles a Bass/Tile kernel, shards the N (query)
dimension across 8 NeuronCores, runs SPMD via run_bass_kernel_spmd, and
gathers the full outputs.

Device algorithm (per core, rows sharded, [m-partitions, q-free] orientation):
  - softmax over m is invariant to per-q factors, so exp(logits) factors as
      U_h = [exp(T*scores_h/sqrt(Dh)) * exp(-dym)] * exp(0.5*G - 0.25*|xyz_m|^2)
            \______ M_E (computed once) ________/   \_ e' (per iter, ACT) _/
    with G = xyz_m . xyz_q (tiny fp32 matmul) and dym = delta_y where valid
    else 200 (exp -> exactly 0 kills masked pairs). The dropped per-q factor
    exp(-0.25|xyz_q|^2) cancels in the softmax normalization.
  - per iteration x chunk: G matmul -> ACT exp(bias per partition) -> DVE
    U = M_E * e' (bf16) -> PE matmuls with stationary [v_h | xyz | 1] blocks,
    accumulating feat/coords/rowsum per head in PSUM.
  - head normalization/combination uses PE selector-matmuls (partition
    redistribution on DVE is only HW-safe for 32-aligned windows).
  - xyz update allgathered ([L,3] per core) between iterations.
"""

import numpy as np

HEADS = 8
BETA = 0.5
T = 1.0
BAND_WIDTH = 1.0
MAX_ITER = 3
DYM_MASKED = 200.0  # exp(-200) == 0 in fp32/bf16

_cache = {}


def _build(N, D, R):
    """Build + compile the Bacc module. Returns nc."""
    import concourse.bacc as bacc
    import concourse.mybir as mybir
    import concourse.tile as tile

    f32 = mybir.dt.float32
    bf16 = mybir.dt.bfloat16
    Exp = mybir.ActivationFunctionType.Exp
    AX = mybir.AxisListType.X

    H = HEADS
    Dh = D // H                      # 32
    L = N // R                       # local q rows (384)
    C = N // 128                     # m chunks (24)
    CL = L // 128                    # local chunks (3)
    LH = L * H
    W = Dh + 4                       # 36: [v_h | xyz | 1] lhsT columns
    HW_ = H * W                      # 288
    NS = N // 512                    # 512-wide m slices (6)
    SC_SCALE = T / float(np.sqrt(np.float32(Dh)))
    GB = BETA / (2.0 * BAND_WIDTH * BAND_WIDTH)   # 0.25: weight on d2

    nc = bacc.Bacc("TRN2", target_bir_lowering=False, debug=False,
                   num_devices=R)

    din = {}
    def dram_in(name, shape, dtype):
        din[name] = nc.dram_tensor(name, list(shape), dtype, kind="ExternalInput")
        return din[name]

    xT_d = dram_in("xT", [D, N], bf16)            # x^T replicated
    xlocTb_d = dram_in("xlocTb", [D, L], bf16)    # x[rows]^T
    xlocTf_d = dram_in("xlocTf", [D, L], f32)
    dymT_d = dram_in("dymT", [N, L], bf16)        # masked delta_y, transposed
    xyzg0_d = dram_in("xyzg0", [R, 6 * L], f32)   # initial xyz, allgather layout
    xyzlocT0_d = dram_in("xyzlocT0", [3, L], f32)
    wq_d = dram_in("wq", [D, D], bf16)
    wk_d = dram_in("wk", [D, D], bf16)
    wv_d = dram_in("wv", [D, D], bf16)
    wo_d = dram_in("wo", [D, D], bf16)
    negi_d = dram_in("negi", [128, 128], mybir.dt.bfloat16)  # -2*I
    selg_d = dram_in("selg", [128, 32], f32)      # s-row gather pattern
    selh_d = dram_in("selh", [8, 512], f32)       # recip scatter pattern
    selp_d = dram_in("selp", [128, 4], f32)       # head-sum pattern
    i3_d = dram_in("i3", [3, 3], f32)

    outT_d = nc.dram_tensor("outT", [D, L], f32, kind="ExternalOutput")
    xyzout_d = nc.dram_tensor("xyzout", [L, 3], f32, kind="ExternalOutput")

    with tile.TileContext(nc) as tc:
        with (
            tc.tile_pool(name="const", bufs=1) as cp,
            tc.tile_pool(name="dram", bufs=1, space="DRAM") as dp,
            tc.tile_pool(name="xts", bufs=2) as xts_p,
            tc.tile_pool(name="kts", bufs=2) as kts_p,
            tc.tile_pool(name="dymz", bufs=3) as dymz_p,
            tc.tile_pool(name="ez", bufs=2) as ez_p,
            tc.tile_pool(name="u4", bufs=3) as u4_p,
            tc.tile_pool(name="xyzts", bufs=2) as xyzts_p,
            tc.tile_pool(name="fxyz", bufs=2) as fxyz_p,
            tc.tile_pool(name="scb", bufs=2) as scb_p,
        ):
            # ---- persistent SBUF tiles ----
            ME = cp.tile([128, C * LH], bf16)        # masked exp(scores)
            vw = cp.tile([128, C * HW_], bf16)       # [v_h | xyz | 1] blocks
            qT = cp.tile([128, 2 * L], bf16)
            xlocTb = cp.tile([128, 2 * L], bf16)
            xlocTf = cp.tile([128, 2 * L], f32)
            wq_t = cp.tile([128, 2 * D], bf16)
            wk_t = cp.tile([128, 2 * D], bf16)
            wv_t = cp.tile([128, 2 * D], bf16)
            wo_t = cp.tile([128, 2 * D], bf16)
            natf32 = cp.tile([128, 3 * C], f32)
            sqt = cp.tile([128, 3 * C], f32)
            negnorm = cp.tile([128, C], f32)
            natloc = cp.tile([128, 3 * CL], f32)
            xyzlocT = cp.tile([4, L], f32)
            rsP = cp.tile([128, 4 * L], f32)         # per-bank s/xyz row copies
            scaledb = cp.tile([128, L], f32)         # per-bank normalized rows
            recipS8 = cp.tile([8, L], f32)
            xyzlocT67 = cp.tile([67, L], bf16)       # hi/lo split of xyzlocT
            negi = cp.tile([128, 128], bf16)
            selg = cp.tile([128, 32], f32)
            selh = cp.tile([8, 512], f32)
            selp = cp.tile([128, 4], f32)
            i3sb = cp.tile([3, 3], f32)

            agin = dp.tile([1, 6 * L], f32)
            agout = dp.tile([R, 6 * L], f32)

            dma = nc.sync.dma_start
            dma(out=xlocTb[:, :].rearrange("p (k l) -> p k l", l=L),
                in_=xlocTb_d[:, :].rearrange("(k p) l -> p k l", p=128))
            dma(out=xlocTf[:, :].rearrange("p (k l) -> p k l", l=L),
                in_=xlocTf_d[:, :].rearrange("(k p) l -> p k l", p=128))
            for wt, wd in ((wq_t, wq_d), (wk_t, wk_d), (wv_t, wv_d), (wo_t, wo_d)):
                dma(out=wt[:, :].rearrange("p (k d) -> p k d", d=D),
                    in_=wd[:, :].rearrange("(k p) d -> p k d", p=128))
            dma(out=negi[:, :], in_=negi_d[:, :])
            dma(out=selg[:, :], in_=selg_d[:, :])
            dma(out=selh[:, :], in_=selh_d[:, :])
            dma(out=selp[:, :], in_=selp_d[:, :])
            dma(out=i3sb[:, :], in_=i3_d[:, :])
            dma(out=xyzlocT[0:3, :], in_=xyzlocT0_d[:, :])
            # rsP/scaledb rows outside the written ones are read (x0 weight)
            # by the selector matmuls -> must be finite.
            nc.vector.memset(rsP[:, :], 0.0)
            nc.vector.memset(scaledb[:, :], 0.0)

            # ---- phase A PSUM pools (closed before the iteration pools open
            # so the 8 banks can be re-used) ----
            from contextlib import ExitStack
            pa_stack = ExitStack()
            psS_p = pa_stack.enter_context(
                tc.tile_pool(name="psS", bufs=1, space="PSUM"))
            psK_p = pa_stack.enter_context(
                tc.tile_pool(name="psK", bufs=4, space="PSUM"))

            # ---- qT = Wq^T @ xloc^T ----
            for oc in range(2):
                psq = psK_p.tile([128, 512], f32, tag="psk", name="psq")
                for kc in range(2):
                    nc.tensor.matmul(psq[:, 0:L],
                                     lhsT=wq_t[:, D * kc + 128 * oc: D * kc + 128 * oc + 128],
                                     rhs=xlocTb[:, L * kc: L * kc + L],
                                     start=(kc == 0), stop=(kc == 1))
                nc.vector.tensor_copy(qT[:, L * oc: L * oc + L], psq[:, 0:L])

            # ---- phase A ----
            for s in range(NS):
                xts = xts_p.tile([128, 1024], bf16, tag="xts")
                dma(out=xts[:, :].rearrange("p (k m) -> p k m", m=512),
                    in_=xT_d[:, 512 * s: 512 * s + 512]
                        .rearrange("(k p) m -> p k m", p=128))
                kts = kts_p.tile([128, 1024], bf16, tag="kts")
                for oc in range(2):
                    psk = psK_p.tile([128, 512], f32, tag="psk", name="psk")
                    for kc in range(2):
                        nc.tensor.matmul(psk[:, :],
                                         lhsT=wk_t[:, D * kc + 128 * oc: D * kc + 128 * oc + 128],
                                         rhs=xts[:, 512 * kc: 512 * kc + 512],
                                         start=(kc == 0), stop=(kc == 1))
                    nc.vector.tensor_copy(kts[:, 512 * oc: 512 * oc + 512], psk[:, :])
                for cc in range(4):
                    c = 4 * s + cc
                    psv = psK_p.tile([128, 512], f32, tag="psk", name="psv")
                    for kc in range(2):
                        nc.tensor.matmul(psv[:, 0:D],
                                         lhsT=xts[:, 512 * kc + 128 * cc: 512 * kc + 128 * cc + 128],
                                         rhs=wv_t[:, D * kc: D * kc + D],
                                         start=(kc == 0), stop=(kc == 1))
                    nc.vector.tensor_copy(
                        vw[:, HW_ * c: HW_ * c + HW_]
                            .rearrange("p (h w) -> p h w", w=W)[:, :, 0:Dh],
                        psv[:, 0:D].rearrange("p (h j) -> p h j", j=Dh))
                    nc.vector.memset(
                        vw[:, HW_ * c: HW_ * c + HW_]
                            .rearrange("p (h w) -> p h w", w=W)[:, :, Dh + 3:Dh + 4],
                        1.0)
                for cc in range(4):
                    c = 4 * s + cc
                    for g in range(2):
                        psS = psS_p.tile([128, 2048], f32, tag="pss", name="psS")
                        for i in range(4):
                            nc.tensor.matmul(
                                psS[:, 512 * i: 512 * i + L],
                                lhsT=kts[32 * i: 32 * i + 32,
                                         512 * g + 128 * cc: 512 * g + 128 * cc + 128],
                                rhs=qT[32 * i: 32 * i + 32, L * g: L * g + L],
                                start=True, stop=True,
                                tile_position=(32 * i, 0))
                        nc.scalar.activation(
                            ME[:, LH * c + 4 * L * g: LH * c + 4 * L * g + 4 * L]
                                .rearrange("p (h q) -> p h q", q=L),
                            psS[:, :].rearrange("p (b q) -> p b q", q=512)[:, :, 0:L],
                            Exp, scale=SC_SCALE)

            pa_stack.close()
            psG_p = tc.tile_pool(name="psG", bufs=2, space="PSUM")
            psP_p = tc.tile_pool(name="psP", bufs=4, space="PSUM")
            psM_p = tc.tile_pool(name="psM", bufs=2, space="PSUM")
            it_stack = ExitStack()
            psG_p = it_stack.enter_context(psG_p)
            psP_p = it_stack.enter_context(psP_p)
            psM_p = it_stack.enter_context(psM_p)

            # ---- geometry prep (runs before each iteration's chunks) ----
            def prep_geometry(src):
                for r in range(R):
                    dma(out=natf32[:, 3 * CL * r: 3 * CL * (r + 1)]
                            .rearrange("p (t d) -> p t d", d=3),
                        in_=src[r:r + 1, 0:3 * L]
                            .rearrange("o (t p d) -> p (o t) d", p=128, d=3))
                # hi/lo bf16 split of the local xyz^T for the G matmuls.
                # row pairing with the lhsT side: 0-2 hi*hi, 32-34 hi_m*lo_q,
                # 64-66 lo_m*hi_q (lo*lo is negligible).
                nc.vector.memset(xyzlocT67[0:64, :], 0.0)
                nc.vector.tensor_copy(xyzlocT67[0:3, :], xyzlocT[0:3, :])
                nc.vector.tensor_sub(xyzlocT67[32:35, :], xyzlocT[0:3, :],
                                     xyzlocT67[0:3, :])
                nc.vector.tensor_copy(xyzlocT67[64:67, :], xyzlocT67[0:3, :])
                nc.vector.tensor_mul(sqt[:, :], natf32[:, :], natf32[:, :])
                nc.vector.tensor_reduce(
                    negnorm[:, :].rearrange("p (c o) -> p c o", o=1),
                    sqt[:, :].rearrange("p (c d) -> p c d", d=3),
                    axis=AX, op=mybir.AluOpType.add)
                nc.vector.tensor_scalar_mul(negnorm[:, :], negnorm[:, :], -GB)
                for h in range(H):
                    nc.vector.tensor_copy(
                        vw[:, :].rearrange("p (c h w) -> p c h w", h=H, w=W)
                            [:, :, h, Dh:Dh + 3],
                        natf32[:, :].rearrange("p (c d) -> p c d", d=3))

            prep_geometry(xyzg0_d)

            # ---- iterations ----
            for it in range(MAX_ITER):
                last = (it == MAX_ITER - 1)

                psP = [psP_p.tile([128, 512], f32, tag="psp",
                                  name=f"psP_{it}_{bb}") for bb in range(4)]

                xyzts = None
                for c in range(C):
                    rb, t = divmod(c, CL)
                    if t == 0:
                        fxyz = fxyz_p.tile([3, L], f32, tag="fxyz")
                        src = xyzg0_d if it == 0 else agout
                        dma(out=fxyz[:, :],
                            in_=src[rb:rb + 1, 3 * L: 6 * L]
                                .rearrange("o (d q) -> (o d) q", q=L))
                        xyzts = xyzts_p.tile([67, L], bf16, tag="xyzts")
                        nc.vector.memset(xyzts[0:64, :], 0.0)
                        nc.vector.tensor_copy(xyzts[0:3, :], fxyz[:, :])
                        nc.vector.tensor_copy(xyzts[32:35, :], xyzts[0:3, :])
                        nc.vector.tensor_sub(xyzts[64:67, :], fxyz[:, :],
                                             xyzts[0:3, :])
                    zt = dymz_p.tile([128, L], bf16, tag="dymz")
                    dma(out=zt[:, :], in_=dymT_d[128 * c: 128 * c + 128, :])
                    psG = psG_p.tile([128, 512], f32, tag="psg")
                    nc.tensor.matmul(psG[:, 0:L],
                                     lhsT=xyzts[0:67, 128 * t: 128 * t + 128],
                                     rhs=xyzlocT67[0:67, :],
                                     start=True, stop=False)
                    nc.tensor.matmul(psG[:, 0:L],
                                     lhsT=negi[:, :], rhs=zt[:, :],
                                     start=False, stop=True)
                    et = ez_p.tile([128, 2 * L], bf16, tag="ez")
                    nc.scalar.activation(et[:, 0:L], psG[:, 0:L], Exp,
                                         bias=negnorm[:, c:c + 1],
                                         scale=2.0 * GB)
                    nc.vector.tensor_copy(et[:, L:2 * L], et[:, 0:L])
                    for pair in range(4):
                        u4 = u4_p.tile([128, 2 * L], bf16, tag="u4")
                        nc.vector.tensor_mul(
                            u4[:, :],
                            ME[:, LH * c + 2 * L * pair: LH * c + 2 * L * pair + 2 * L],
                            et[:, :])
                        for i in range(2):
                            h = 2 * pair + i
                            b, hi = h % 4, h // 4
                            nc.tensor.matmul(
                                psP[b][64 * hi: 64 * hi + W, 0:L],
                                lhsT=vw[:, HW_ * c + W * h: HW_ * c + W * h + W],
                                rhs=u4[:, L * i: L * i + L],
                                start=(c == 0), stop=(c == C - 1),
                                skip_group_check=True)

                # ---- boundary: normalize heads, update xyz ----
                # psP rows per 64-block: feat@0-31, xyz@32-34, s@35.
                for b in range(4):
                    for hi in range(2):
                        p0 = 64 * hi + 32
                        nc.vector.tensor_copy(rsP[p0:p0 + 4, L * b: L * b + L],
                                              psP[b][p0:p0 + 4, 0:L])
                # gather the 8 s rows -> [8, L], one batched reciprocal,
                # then scatter 1/s (and 1/(H s)) back per bank.
                psS8 = psM_p.tile([128, 512], f32, tag="psm", name="psS8")
                for b in range(4):
                    nc.tensor.matmul(psS8[0:8, 0:L],
                                     lhsT=selg[:, 8 * b: 8 * b + 8],
                                     rhs=rsP[:, L * b: L * b + L],
                                     start=(b == 0), stop=(b == 3))
                nc.vector.reciprocal(recipS8[0:8, :], psS8[0:8, 0:L])
                psX = psM_p.tile([128, 512], f32, tag="psm", name="psX")
                for b in range(4):
                    psR = psM_p.tile([128, 512], f32, tag="psm", name="psR")
                    nc.tensor.matmul(psR[:, 0:L],
                                     lhsT=selh[:, 128 * b: 128 * b + 128],
                                     rhs=recipS8[0:8, :],
                                     start=True, stop=True)
                    rsb = scb_p.tile([128, L], f32, tag="scb")
                    nc.vector.tensor_copy(rsb[:, :], psR[:, 0:L])
                    # only the matmul-written PSUM rows are touched (elsewhere
                    # PSUM may hold non-finite garbage); scaledb stays 0 there.
                    for hi in range(2):
                        p0 = 64 * hi
                        nc.vector.tensor_mul(scaledb[p0:p0 + W, :],
                                             psP[b][p0:p0 + W, 0:L],
                                             rsb[p0:p0 + W, :])
                    nc.tensor.matmul(psX[0:4, 0:L],
                                     lhsT=selp[:, :],
                                     rhs=scaledb[:, :],
                                     start=(b == 0), stop=(b == 3))
                    if last:
                        for hi in range(2):
                            nc.vector.tensor_copy(
                                qT[32 * b: 32 * b + 32, L * hi: L * hi + L],
                                scaledb[64 * hi: 64 * hi + 32, :])
                nc.vector.tensor_copy(xyzlocT[0:4, :], psX[0:4, 0:L])

                for t in range(CL):
                    psN = psM_p.tile([128, 512], f32, tag="psm")
                    nc.tensor.matmul(psN[:, 0:3],
                                     lhsT=xyzlocT[0:3, 128 * t: 128 * t + 128],
                                     rhs=i3sb[:, :], start=True, stop=True)
                    nc.vector.tensor_copy(natloc[:, 3 * t: 3 * t + 3], psN[:, 0:3])

                if last:
                    dma(out=xyzout_d[:, :].rearrange("(t p) d -> p t d", p=128),
                        in_=natloc[:, :].rearrange("p (t d) -> p t d", d=3))
                    for oc in range(2):
                        psO = psM_p.tile([128, 512], f32, tag="psm")
                        for kc in range(2):
                            nc.tensor.matmul(
                                psO[:, 0:L],
                                lhsT=wo_t[:, D * kc + 128 * oc: D * kc + 128 * oc + 128],
                                rhs=qT[:, L * kc: L * kc + L],
                                start=(kc == 0), stop=(kc == 1))
                        nc.vector.tensor_add(xlocTf[:, L * oc: L * oc + L],
                                             psO[:, 0:L],
                                             xlocTf[:, L * oc: L * oc + L])
                        dma(out=outT_d[128 * oc: 128 * oc + 128, :],
                            in_=xlocTf[:, L * oc: L * oc + L])
                else:
                    dma(out=agin[0:1, 0:3 * L].rearrange("o (t p d) -> p (o t) d",
                                                         p=128, d=3),
                        in_=natloc[:, :].rearrange("p (t d) -> p t d", d=3))
                    dma(out=agin[0:1, 3 * L: 6 * L]
                            .rearrange("o (d q) -> (o d) q", q=L),
                        in_=xyzlocT[0:3, :])
                    nc.gpsimd.collective_compute(
                        "AllGather", mybir.AluOpType.bypass,
                        replica_groups=[list(range(R))],
                        ins=[agin[:, :].opt()], outs=[agout[:, :].opt()])
                    prep_geometry(agout)

            it_stack.close()

    nc.compile()
    return nc


def _get_nc(N, D, R):
    key = (N, D, R)
    if key not in _cache:
        _cache[key] = _build(N, D, R)
    return _cache[key]


def _make_consts(H):
    """Selector matmul patterns for head normalization (see boundary code)."""
    selg = np.zeros((128, 32), np.float32)    # gather s rows -> [8, L]
    selh = np.zeros((8, 512), np.float32)     # scatter 1/s back per bank
    selp = np.zeros((128, 4), np.float32)     # sum xyz rows across heads
    for b in range(4):
        for hi in range(2):
            h = 4 * hi + b
            selg[64 * hi + 35, 8 * b + h] = 1.0
            for j in range(32):
                selh[h, 128 * b + 64 * hi + j] = 1.0
            for c in range(3):
                selh[h, 128 * b + 64 * hi + 32 + c] = 1.0 / H
    for hi in range(2):
        for c in range(3):
            selp[64 * hi + 32 + c, c] = 1.0
    return selg, selh, selp


def _host_prep(x, xyz, delta_y, distance_mask, big_inter_mask,
               Wq, bq, Wk, bk, Wv, bv, Wo, bo, R):
    import concourse.mybir as mybir
    bf = mybir.dt.np(mybir.dt.bfloat16)
    N, D = x.shape
    L = N // R

    for b in (bq, bk, bv, bo):
        assert np.max(np.abs(np.asarray(b))) == 0.0, "nonzero biases unsupported"

    x = np.asarray(x, np.float32)
    xyz = np.asarray(xyz, np.float32)
    delta_y = np.asarray(delta_y, np.float32)
    valid = (np.asarray(distance_mask, bool) & np.asarray(big_inter_mask, bool))
    np.fill_diagonal(valid, True)
    dym = np.where(valid, delta_y, DYM_MASKED).astype(np.float32)

    xT_bf = np.ascontiguousarray(x.T).astype(bf)
    wq_bf = np.ascontiguousarray(np.asarray(Wq, np.float32)).astype(bf)
    wk_bf = np.ascontiguousarray(np.asarray(Wk, np.float32)).astype(bf)
    wv_bf = np.ascontiguousarray(np.asarray(Wv, np.float32)).astype(bf)
    wo_bf = np.ascontiguousarray(np.asarray(Wo, np.float32)).astype(bf)

    xyzg0 = np.zeros((R, 6 * L), np.float32)
    for r in range(R):
        blk = xyz[L * r: L * r + L]
        xyzg0[r, 0:3 * L] = blk.reshape(-1)
        xyzg0[r, 3 * L:6 * L] = np.ascontiguousarray(blk.T).reshape(-1)

    selg, selh, selp = _make_consts(HEADS)
    i3 = np.eye(3, dtype=np.float32)
    negi = (-2.0 * np.eye(128, dtype=np.float32)).astype(bf)

    in_maps = []
    for r in range(R):
        rows = slice(L * r, L * (r + 1))
        in_maps.append(dict(
            xT=xT_bf,
            xlocTb=np.ascontiguousarray(x[rows].T).astype(bf),
            xlocTf=np.ascontiguousarray(x[rows].T),
            dymT=np.ascontiguousarray(dym[rows].T).astype(bf),
            xyzg0=xyzg0,
            xyzlocT0=np.ascontiguousarray(xyz[rows].T),
            wq=wq_bf, wk=wk_bf, wv=wv_bf, wo=wo_bf,
            selg=selg, selh=selh, selp=selp, i3=i3, negi=negi,
        ))
    return in_maps


def run(inputs, R=8, trace=False):
    from concourse.bass_utils import run_bass_kernel_spmd
    N, D = inputs["x"].shape
    nc = _get_nc(N, D, R)
    in_maps = _host_prep(R=R, **inputs)
    res = run_bass_kernel_spmd(nc, in_maps, list(range(R)), trace=trace)
    L = N // R
    out = np.zeros((N, D), np.float32)
    xyz_out = np.zeros((N, 3), np.float32)
    for r in range(R):
        out[L * r: L * r + L] = np.asarray(res.results[r]["outT"], np.float32).T
        xyz_out[L * r: L * r + L] = np.asarray(res.results[r]["xyzout"], np.float32)
    return (xyz_out, out), res


def kernel(**inputs):
    (xyz_out, out), _ = run(inputs, R=8, trace=False)
    return xyz_out, out


# revision 26
# speedup vs baseline: 1.0511x; 1.0511x over previous
"""Trainium2 Bass kernel for GAT_MS_2_3 (iterative mean-shift sparse attention).

Self-contained: builds + compiles a Bass/Tile kernel, shards the N (query)
dimension across 8 NeuronCores, runs SPMD via run_bass_kernel_spmd, and
gathers the full outputs.

Device algorithm (per core, rows sharded, [m-partitions, q-free] orientation):
  - softmax over m is invariant to per-q factors, so exp(logits) factors as
      U_h = [exp(T*scores_h/sqrt(Dh)) * exp(-dym)] * exp(0.5*G - 0.25*|xyz_m|^2)
            \______ M_E (computed once) ________/   \_ e' (per iter, ACT) _/
    with G = xyz_m . xyz_q (tiny fp32 matmul) and dym = delta_y where valid
    else 200 (exp -> exactly 0 kills masked pairs). The dropped per-q factor
    exp(-0.25|xyz_q|^2) cancels in the softmax normalization.
  - per iteration x chunk: G matmul -> ACT exp(bias per partition) -> DVE
    U = M_E * e' (bf16) -> PE matmuls with stationary [v_h | xyz | 1] blocks,
    accumulating feat/coords/rowsum per head in PSUM.
  - head normalization/combination uses PE selector-matmuls (partition
    redistribution on DVE is only HW-safe for 32-aligned windows).
  - xyz update allgathered ([L,3] per core) between iterations.
"""

import numpy as np

HEADS = 8
BETA = 0.5
T = 1.0
BAND_WIDTH = 1.0
MAX_ITER = 3
DYM_MASKED = 200.0  # exp(-200) == 0 in fp32/bf16

_cache = {}


def _build(N, D, R):
    """Build + compile the Bacc module. Returns nc."""
    import concourse.bacc as bacc
    import concourse.mybir as mybir
    import concourse.tile as tile

    f32 = mybir.dt.float32
    bf16 = mybir.dt.bfloat16
    Exp = mybir.ActivationFunctionType.Exp
    AX = mybir.AxisListType.X

    H = HEADS
    Dh = D // H                      # 32
    L = N // R                       # local q rows (384)
    C = N // 128                     # m chunks (24)
    CL = L // 128                    # local chunks (3)
    LH = L * H
    W = Dh + 4                       # 36: [v_h | xyz | 1] lhsT columns
    HW_ = H * W                      # 288
    NS = N // 512                    # 512-wide m slices (6)
    SC_SCALE = T / float(np.sqrt(np.float32(Dh)))
    GB = BETA / (2.0 * BAND_WIDTH * BAND_WIDTH)   # 0.25: weight on d2

    nc = bacc.Bacc("TRN2", target_bir_lowering=False, debug=False,
                   num_devices=R)

    din = {}
    def dram_in(name, shape, dtype):
        din[name] = nc.dram_tensor(name, list(shape), dtype, kind="ExternalInput")
        return din[name]

    xT_d = dram_in("xT", [D, N], bf16)            # x^T replicated
    xlocTb_d = dram_in("xlocTb", [D, L], bf16)    # x[rows]^T
    xlocTf_d = dram_in("xlocTf", [D, L], f32)
    dymT_d = dram_in("dymT", [N, L], bf16)        # masked delta_y, transposed
    xyzg0_d = dram_in("xyzg0", [R, 6 * L], f32)   # initial xyz, allgather layout
    xyzlocT0_d = dram_in("xyzlocT0", [3, L], f32)
    wq_d = dram_in("wq", [D, D], bf16)
    wk_d = dram_in("wk", [D, D], bf16)
    wv_d = dram_in("wv", [D, D], bf16)
    wo_d = dram_in("wo", [D, D], bf16)
    negi_d = dram_in("negi", [128, 128], mybir.dt.bfloat16)  # -2*I
    selg_d = dram_in("selg", [128, 32], f32)      # s-row gather pattern
    selh_d = dram_in("selh", [8, 512], f32)       # recip scatter pattern
    selp_d = dram_in("selp", [128, 4], f32)       # head-sum pattern
    i3_d = dram_in("i3", [3, 3], f32)

    outT_d = nc.dram_tensor("outT", [D, L], f32, kind="ExternalOutput")
    xyzout_d = nc.dram_tensor("xyzout", [L, 3], f32, kind="ExternalOutput")

    with tile.TileContext(nc) as tc:
        with (
            tc.tile_pool(name="const", bufs=1) as cp,
            tc.tile_pool(name="dram", bufs=1, space="DRAM") as dp,
            tc.tile_pool(name="xts", bufs=2) as xts_p,
            tc.tile_pool(name="kts", bufs=2) as kts_p,
            tc.tile_pool(name="dymz", bufs=3) as dymz_p,
            tc.tile_pool(name="ez", bufs=2) as ez_p,
            tc.tile_pool(name="u4", bufs=3) as u4_p,
            tc.tile_pool(name="xyzts", bufs=2) as xyzts_p,
            tc.tile_pool(name="fxyz", bufs=2) as fxyz_p,
            tc.tile_pool(name="scb", bufs=2) as scb_p,
        ):
            # ---- persistent SBUF tiles ----
            ME = cp.tile([128, C * LH], bf16)        # masked exp(scores)
            vw = cp.tile([128, C * HW_], bf16)       # [v_h | xyz | 1] blocks
            qT = cp.tile([128, 2 * L], bf16)
            xlocTb = cp.tile([128, 2 * L], bf16)
            xlocTf = cp.tile([128, 2 * L], f32)
            wq_t = cp.tile([128, 2 * D], bf16)
            wk_t = cp.tile([128, 2 * D], bf16)
            wv_t = cp.tile([128, 2 * D], bf16)
            wo_t = cp.tile([128, 2 * D], bf16)
            natf32 = cp.tile([128, 3 * C], f32)
            sqt = cp.tile([128, 3 * C], f32)
            negnorm = cp.tile([128, C], f32)
            natloc = cp.tile([128, 3 * CL], f32)
            xyzlocT = cp.tile([4, L], f32)
            rsP = cp.tile([128, 4 * L], f32)         # per-bank s/xyz row copies
            scaledb = cp.tile([128, L], f32)         # per-bank normalized rows
            recipS8 = cp.tile([8, L], f32)
            xyzlocT67 = cp.tile([67, L], bf16)       # hi/lo split of xyzlocT
            negi = cp.tile([128, 128], bf16)
            selg = cp.tile([128, 32], f32)
            selh = cp.tile([8, 512], f32)
            selp = cp.tile([128, 4], f32)
            i3sb = cp.tile([3, 3], f32)

            agin = dp.tile([1, 6 * L], f32)
            agout = dp.tile([R, 6 * L], f32)

            dma = nc.sync.dma_start
            dma(out=xlocTb[:, :].rearrange("p (k l) -> p k l", l=L),
                in_=xlocTb_d[:, :].rearrange("(k p) l -> p k l", p=128))
            dma(out=xlocTf[:, :].rearrange("p (k l) -> p k l", l=L),
                in_=xlocTf_d[:, :].rearrange("(k p) l -> p k l", p=128))
            for wt, wd in ((wq_t, wq_d), (wk_t, wk_d), (wv_t, wv_d), (wo_t, wo_d)):
                dma(out=wt[:, :].rearrange("p (k d) -> p k d", d=D),
                    in_=wd[:, :].rearrange("(k p) d -> p k d", p=128))
            dma(out=negi[:, :], in_=negi_d[:, :])
            dma(out=selg[:, :], in_=selg_d[:, :])
            dma(out=selh[:, :], in_=selh_d[:, :])
            dma(out=selp[:, :], in_=selp_d[:, :])
            dma(out=i3sb[:, :], in_=i3_d[:, :])
            dma(out=xyzlocT[0:3, :], in_=xyzlocT0_d[:, :])
            # rsP/scaledb rows outside the written ones are read (x0 weight)
            # by the selector matmuls -> must be finite.
            nc.vector.memset(rsP[:, :], 0.0)
            nc.vector.memset(scaledb[:, :], 0.0)

            # ---- phase A PSUM pools (closed before the iteration pools open
            # so the 8 banks can be re-used) ----
            from contextlib import ExitStack
            pa_stack = ExitStack()
            psS_p = pa_stack.enter_context(
                tc.tile_pool(name="psS", bufs=2, space="PSUM"))

            # ---- qT = Wq^T @ xloc^T ----
            psq = psS_p.tile([128, 2048], f32, tag="pss", name="psq")
            for oc in range(2):
                for kc in range(2):
                    nc.tensor.matmul(psq[:, 512 * oc: 512 * oc + L],
                                     lhsT=wq_t[:, D * kc + 128 * oc: D * kc + 128 * oc + 128],
                                     rhs=xlocTb[:, L * kc: L * kc + L],
                                     start=(kc == 0), stop=(kc == 1))
            nc.vector.tensor_copy(
                qT[:, :].rearrange("p (o l) -> p o l", l=L),
                psq[:, 0:1024].rearrange("p (o x) -> p o x", x=512)[:, :, 0:L])

            # ---- phase A ----
            for s in range(NS):
                xts = xts_p.tile([128, 1024], bf16, tag="xts")
                dma(out=xts[:, :].rearrange("p (k m) -> p k m", m=512),
                    in_=xT_d[:, 512 * s: 512 * s + 512]
                        .rearrange("(k p) m -> p k m", p=128))
                kts = kts_p.tile([128, 1024], bf16, tag="kts")
                psk = psS_p.tile([128, 2048], f32, tag="pss", name="psk")
                for oc in range(2):
                    for kc in range(2):
                        nc.tensor.matmul(psk[:, 512 * oc: 512 * oc + 512],
                                         lhsT=wk_t[:, D * kc + 128 * oc: D * kc + 128 * oc + 128],
                                         rhs=xts[:, 512 * kc: 512 * kc + 512],
                                         start=(kc == 0), stop=(kc == 1))
                nc.vector.tensor_copy(kts[:, :], psk[:, 0:1024])
                psv = psS_p.tile([128, 2048], f32, tag="pss", name="psv")
                for cc in range(4):
                    for kc in range(2):
                        nc.tensor.matmul(psv[:, 512 * cc: 512 * cc + D],
                                         lhsT=xts[:, 512 * kc + 128 * cc: 512 * kc + 128 * cc + 128],
                                         rhs=wv_t[:, D * kc: D * kc + D],
                                         start=(kc == 0), stop=(kc == 1))
                nc.vector.tensor_copy(
                    vw[:, HW_ * 4 * s: HW_ * 4 * s + 4 * HW_]
                        .rearrange("p (c h w) -> p c h w", h=H, w=W)[:, :, :, 0:Dh],
                    psv[:, :].rearrange("p (c x) -> p c x", x=512)
                        [:, :, 0:D].rearrange("p c (h j) -> p c h j", j=Dh))
                nc.vector.memset(
                    vw[:, HW_ * 4 * s: HW_ * 4 * s + 4 * HW_]
                        .rearrange("p (c h w) -> p c h w", h=H, w=W)
                        [:, :, :, Dh + 3:Dh + 4],
                    1.0)
                fp16 = mybir.dt.float16
                for cc in range(4):
                    c = 4 * s + cc
                    for g in range(2):
                        psS = psS_p.tile([128, 2048], f32, tag="pss", name="psS")
                        for i in range(4):
                            nc.tensor.matmul(
                                psS[:, 512 * i: 512 * i + L],
                                lhsT=kts[32 * i: 32 * i + 32,
                                         512 * g + 128 * cc: 512 * g + 128 * cc + 128],
                                rhs=qT[32 * i: 32 * i + 32, L * g: L * g + L],
                                start=True, stop=True,
                                tile_position=(32 * i, 0))
                        nc.vector.tensor_copy(
                            ME[:, LH * c + 4 * L * g: LH * c + 4 * L * g + 4 * L]
                                .bitcast(fp16).rearrange("p (h q) -> p h q", q=L),
                            psS[:, :].rearrange("p (b q) -> p b q", q=512)[:, :, 0:L])
                    nc.scalar.activation(
                        ME[:, LH * c: LH * c + LH],
                        ME[:, LH * c: LH * c + LH].bitcast(fp16),
                        Exp, scale=SC_SCALE)

            pa_stack.close()
            psG_p = tc.tile_pool(name="psG", bufs=2, space="PSUM")
            psP_p = tc.tile_pool(name="psP", bufs=4, space="PSUM")
            psM_p = tc.tile_pool(name="psM", bufs=2, space="PSUM")
            it_stack = ExitStack()
            psG_p = it_stack.enter_context(psG_p)
            psP_p = it_stack.enter_context(psP_p)
            psM_p = it_stack.enter_context(psM_p)

            # ---- geometry prep (runs before each iteration's chunks) ----
            def prep_geometry(src):
                for r in range(R):
                    dma(out=natf32[:, 3 * CL * r: 3 * CL * (r + 1)]
                            .rearrange("p (t d) -> p t d", d=3),
                        in_=src[r:r + 1, 0:3 * L]
                            .rearrange("o (t p d) -> p (o t) d", p=128, d=3))
                # hi/lo bf16 split of the local xyz^T for the G matmuls.
                # row pairing with the lhsT side: 0-2 hi*hi, 32-34 hi_m*lo_q,
                # 64-66 lo_m*hi_q (lo*lo is negligible).
                nc.vector.memset(xyzlocT67[0:64, :], 0.0)
                nc.vector.tensor_copy(xyzlocT67[0:3, :], xyzlocT[0:3, :])
                nc.vector.tensor_sub(xyzlocT67[32:35, :], xyzlocT[0:3, :],
                                     xyzlocT67[0:3, :])
                nc.vector.tensor_copy(xyzlocT67[64:67, :], xyzlocT67[0:3, :])
                nc.vector.tensor_mul(sqt[:, :], natf32[:, :], natf32[:, :])
                nc.vector.tensor_reduce(
                    negnorm[:, :].rearrange("p (c o) -> p c o", o=1),
                    sqt[:, :].rearrange("p (c d) -> p c d", d=3),
                    axis=AX, op=mybir.AluOpType.add)
                nc.vector.tensor_scalar_mul(negnorm[:, :], negnorm[:, :], -GB)
                for h in range(H):
                    nc.vector.tensor_copy(
                        vw[:, :].rearrange("p (c h w) -> p c h w", h=H, w=W)
                            [:, :, h, Dh:Dh + 3],
                        natf32[:, :].rearrange("p (c d) -> p c d", d=3))

            prep_geometry(xyzg0_d)

            # ---- iterations ----
            for it in range(MAX_ITER):
                last = (it == MAX_ITER - 1)

                psP = [psP_p.tile([128, 512], f32, tag="psp",
                                  name=f"psP_{it}_{bb}") for bb in range(4)]

                xyzts = None
                for c in range(C):
                    rb, t = divmod(c, CL)
                    if t == 0:
                        fxyz = fxyz_p.tile([3, L], f32, tag="fxyz")
                        src = xyzg0_d if it == 0 else agout
                        dma(out=fxyz[:, :],
                            in_=src[rb:rb + 1, 3 * L: 6 * L]
                                .rearrange("o (d q) -> (o d) q", q=L))
                        xyzts = xyzts_p.tile([67, L], bf16, tag="xyzts")
                        nc.vector.memset(xyzts[0:64, :], 0.0)
                        nc.vector.tensor_copy(xyzts[0:3, :], fxyz[:, :])
                        nc.vector.tensor_copy(xyzts[32:35, :], xyzts[0:3, :])
                        nc.vector.tensor_sub(xyzts[64:67, :], fxyz[:, :],
                                             xyzts[0:3, :])
                    zt = dymz_p.tile([128, L], bf16, tag="dymz")
                    dma(out=zt[:, :], in_=dymT_d[128 * c: 128 * c + 128, :])
                    psG = psG_p.tile([128, 512], f32, tag="psg")
                    nc.tensor.matmul(psG[:, 0:L],
                                     lhsT=xyzts[0:67, 128 * t: 128 * t + 128],
                                     rhs=xyzlocT67[0:67, :],
                                     start=True, stop=False)
                    nc.tensor.matmul(psG[:, 0:L],
                                     lhsT=negi[:, :], rhs=zt[:, :],
                                     start=False, stop=True)
                    et = ez_p.tile([128, 2 * L], bf16, tag="ez")
                    nc.scalar.activation(et[:, 0:L], psG[:, 0:L], Exp,
                                         bias=negnorm[:, c:c + 1],
                                         scale=2.0 * GB)
                    nc.vector.tensor_copy(et[:, L:2 * L], et[:, 0:L])
                    for pair in range(4):
                        u4 = u4_p.tile([128, 2 * L], bf16, tag="u4")
                        nc.vector.tensor_mul(
                            u4[:, :],
                            ME[:, LH * c + 2 * L * pair: LH * c + 2 * L * pair + 2 * L],
                            et[:, :])
                        for i in range(2):
                            h = 2 * pair + i
                            b, hi = h % 4, h // 4
                            nc.tensor.matmul(
                                psP[b][64 * hi: 64 * hi + W, 0:L],
                                lhsT=vw[:, HW_ * c + W * h: HW_ * c + W * h + W],
                                rhs=u4[:, L * i: L * i + L],
                                start=(c == 0), stop=(c == C - 1),
                                skip_group_check=True)

                # ---- boundary: normalize heads, update xyz ----
                # psP rows per 64-block: feat@0-31, xyz@32-34, s@35.
                for b in range(4):
                    for hi in range(2):
                        p0 = 64 * hi + 32
                        nc.vector.tensor_copy(rsP[p0:p0 + 4, L * b: L * b + L],
                                              psP[b][p0:p0 + 4, 0:L])
                # gather the 8 s rows -> [8, L], one batched reciprocal,
                # then scatter 1/s (and 1/(H s)) back per bank.
                psS8 = psM_p.tile([128, 512], f32, tag="psm", name="psS8")
                for b in range(4):
                    nc.tensor.matmul(psS8[0:8, 0:L],
                                     lhsT=selg[:, 8 * b: 8 * b + 8],
                                     rhs=rsP[:, L * b: L * b + L],
                                     start=(b == 0), stop=(b == 3))
                nc.vector.reciprocal(recipS8[0:8, :], psS8[0:8, 0:L])
                psX = psM_p.tile([128, 512], f32, tag="psm", name="psX")
                for b in range(4):
                    psR = psM_p.tile([128, 512], f32, tag="psm", name="psR")
                    nc.tensor.matmul(psR[:, 0:L],
                                     lhsT=selh[:, 128 * b: 128 * b + 128],
                                     rhs=recipS8[0:8, :],
                                     start=True, stop=True)
                    rsb = scb_p.tile([128, L], f32, tag="scb")
                    nc.vector.tensor_copy(rsb[:, :], psR[:, 0:L])
                    # only the matmul-written PSUM rows are touched (elsewhere
                    # PSUM may hold non-finite garbage); scaledb stays 0 there.
                    for hi in range(2):
                        p0 = 64 * hi
                        nc.vector.tensor_mul(scaledb[p0:p0 + W, :],
                                             psP[b][p0:p0 + W, 0:L],
                                             rsb[p0:p0 + W, :])
                    nc.tensor.matmul(psX[0:4, 0:L],
                                     lhsT=selp[:, :],
                                     rhs=scaledb[:, :],
                                     start=(b == 0), stop=(b == 3))
                    if last:
                        for hi in range(2):
                            nc.vector.tensor_copy(
                                qT[32 * b: 32 * b + 32, L * hi: L * hi + L],
                                scaledb[64 * hi: 64 * hi + 32, :])
                nc.vector.tensor_copy(xyzlocT[0:4, :], psX[0:4, 0:L])

                for t in range(CL):
                    psN = psM_p.tile([128, 512], f32, tag="psm")
                    nc.tensor.matmul(psN[:, 0:3],
                                     lhsT=xyzlocT[0:3, 128 * t: 128 * t + 128],
                                     rhs=i3sb[:, :], start=True, stop=True)
                    nc.vector.tensor_copy(natloc[:, 3 * t: 3 * t + 3], psN[:, 0:3])

                if last:
                    dma(out=xyzout_d[:, :].rearrange("(t p) d -> p t d", p=128),
                        in_=natloc[:, :].rearrange("p (t d) -> p t d", d=3))
                    for oc in range(2):
                        psO = psM_p.tile([128, 512], f32, tag="psm")
                        for kc in range(2):
                            nc.tensor.matmul(
                                psO[:, 0:L],
                                lhsT=wo_t[:, D * kc + 128 * oc: D * kc + 128 * oc + 128],
                                rhs=qT[:, L * kc: L * kc + L],
                                start=(kc == 0), stop=(kc == 1))
                        nc.vector.tensor_add(xlocTf[:, L * oc: L * oc + L],
                                             psO[:, 0:L],
                                             xlocTf[:, L * oc: L * oc + L])
                        dma(out=outT_d[128 * oc: 128 * oc + 128, :],
                            in_=xlocTf[:, L * oc: L * oc + L])
                else:
                    dma(out=agin[0:1, 0:3 * L].rearrange("o (t p d) -> p (o t) d",
                                                         p=128, d=3),
                        in_=natloc[:, :].rearrange("p (t d) -> p t d", d=3))
                    dma(out=agin[0:1, 3 * L: 6 * L]
                            .rearrange("o (d q) -> (o d) q", q=L),
                        in_=xyzlocT[0:3, :])
                    nc.gpsimd.collective_compute(
                        "AllGather", mybir.AluOpType.bypass,
                        replica_groups=[list(range(R))],
                        ins=[agin[:, :].opt()], outs=[agout[:, :].opt()])
                    prep_geometry(agout)

            it_stack.close()

    nc.compile()
    return nc


def _get_nc(N, D, R):
    key = (N, D, R)
    if key not in _cache:
        _cache[key] = _build(N, D, R)
    return _cache[key]


def _make_consts(H):
    """Selector matmul patterns for head normalization (see boundary code)."""
    selg = np.zeros((128, 32), np.float32)    # gather s rows -> [8, L]
    selh = np.zeros((8, 512), np.float32)     # scatter 1/s back per bank
    selp = np.zeros((128, 4), np.float32)     # sum xyz rows across heads
    for b in range(4):
        for hi in range(2):
            h = 4 * hi + b
            selg[64 * hi + 35, 8 * b + h] = 1.0
            for j in range(32):
                selh[h, 128 * b + 64 * hi + j] = 1.0
            for c in range(3):
                selh[h, 128 * b + 64 * hi + 32 + c] = 1.0 / H
    for hi in range(2):
        for c in range(3):
            selp[64 * hi + 32 + c, c] = 1.0
    return selg, selh, selp


def _host_prep(x, xyz, delta_y, distance_mask, big_inter_mask,
               Wq, bq, Wk, bk, Wv, bv, Wo, bo, R):
    import concourse.mybir as mybir
    bf = mybir.dt.np(mybir.dt.bfloat16)
    N, D = x.shape
    L = N // R

    for b in (bq, bk, bv, bo):
        assert np.max(np.abs(np.asarray(b))) == 0.0, "nonzero biases unsupported"

    x = np.asarray(x, np.float32)
    xyz = np.asarray(xyz, np.float32)
    delta_y = np.asarray(delta_y, np.float32)
    valid = (np.asarray(distance_mask, bool) & np.asarray(big_inter_mask, bool))
    np.fill_diagonal(valid, True)
    dym = np.where(valid, delta_y, DYM_MASKED).astype(np.float32)

    xT_bf = np.ascontiguousarray(x.T).astype(bf)
    wq_bf = np.ascontiguousarray(np.asarray(Wq, np.float32)).astype(bf)
    wk_bf = np.ascontiguousarray(np.asarray(Wk, np.float32)).astype(bf)
    wv_bf = np.ascontiguousarray(np.asarray(Wv, np.float32)).astype(bf)
    wo_bf = np.ascontiguousarray(np.asarray(Wo, np.float32)).astype(bf)

    xyzg0 = np.zeros((R, 6 * L), np.float32)
    for r in range(R):
        blk = xyz[L * r: L * r + L]
        xyzg0[r, 0:3 * L] = blk.reshape(-1)
        xyzg0[r, 3 * L:6 * L] = np.ascontiguousarray(blk.T).reshape(-1)

    selg, selh, selp = _make_consts(HEADS)
    i3 = np.eye(3, dtype=np.float32)
    negi = (-2.0 * np.eye(128, dtype=np.float32)).astype(bf)

    in_maps = []
    for r in range(R):
        rows = slice(L * r, L * (r + 1))
        in_maps.append(dict(
            xT=xT_bf,
            xlocTb=np.ascontiguousarray(x[rows].T).astype(bf),
            xlocTf=np.ascontiguousarray(x[rows].T),
            dymT=np.ascontiguousarray(dym[rows].T).astype(bf),
            xyzg0=xyzg0,
            xyzlocT0=np.ascontiguousarray(xyz[rows].T),
            wq=wq_bf, wk=wk_bf, wv=wv_bf, wo=wo_bf,
            selg=selg, selh=selh, selp=selp, i3=i3, negi=negi,
        ))
    return in_maps


def run(inputs, R=8, trace=False):
    from concourse.bass_utils import run_bass_kernel_spmd
    N, D = inputs["x"].shape
    nc = _get_nc(N, D, R)
    in_maps = _host_prep(R=R, **inputs)
    res = run_bass_kernel_spmd(nc, in_maps, list(range(R)), trace=trace)
    L = N // R
    out = np.zeros((N, D), np.float32)
    xyz_out = np.zeros((N, 3), np.float32)
    for r in range(R):
        out[L * r: L * r + L] = np.asarray(res.results[r]["outT"], np.float32).T
        xyz_out[L * r: L * r + L] = np.asarray(res.results[r]["xyzout"], np.float32)
    return (xyz_out, out), res


def kernel(**inputs):
    (xyz_out, out), _ = run(inputs, R=8, trace=False)
    return xyz_out, out


# revision 28
# speedup vs baseline: 1.0735x; 1.0213x over previous
"""Trainium2 Bass kernel for GAT_MS_2_3 (iterative mean-shift sparse attention).

Self-contained: builds + compiles a Bass/Tile kernel, shards the N (query)
dimension across 8 NeuronCores, runs SPMD via run_bass_kernel_spmd, and
gathers the full outputs.

Device algorithm (per core, rows sharded, [m-partitions, q-free] orientation):
  - softmax over m is invariant to per-q factors, so exp(logits) factors as
      U_h = [exp(T*scores_h/sqrt(Dh)) * exp(-dym)] * exp(0.5*G - 0.25*|xyz_m|^2)
            \______ M_E (computed once) ________/   \_ e' (per iter, ACT) _/
    with G = xyz_m . xyz_q (tiny fp32 matmul) and dym = delta_y where valid
    else 200 (exp -> exactly 0 kills masked pairs). The dropped per-q factor
    exp(-0.25|xyz_q|^2) cancels in the softmax normalization.
  - per iteration x chunk: G matmul -> ACT exp(bias per partition) -> DVE
    U = M_E * e' (bf16) -> PE matmuls with stationary [v_h | xyz | 1] blocks,
    accumulating feat/coords/rowsum per head in PSUM.
  - head normalization/combination uses PE selector-matmuls (partition
    redistribution on DVE is only HW-safe for 32-aligned windows).
  - xyz update allgathered ([L,3] per core) between iterations.
"""

import numpy as np

HEADS = 8
BETA = 0.5
T = 1.0
BAND_WIDTH = 1.0
MAX_ITER = 3
DYM_MASKED = 200.0  # exp(-200) == 0 in fp32/bf16

_cache = {}


def _build(N, D, R):
    """Build + compile the Bacc module. Returns nc."""
    import concourse.bacc as bacc
    import concourse.mybir as mybir
    import concourse.tile as tile

    f32 = mybir.dt.float32
    bf16 = mybir.dt.bfloat16
    Exp = mybir.ActivationFunctionType.Exp
    AX = mybir.AxisListType.X

    H = HEADS
    Dh = D // H                      # 32
    L = N // R                       # local q rows (384)
    C = N // 128                     # m chunks (24)
    CL = L // 128                    # local chunks (3)
    LH = L * H
    W = Dh + 4                       # 36: [v_h | xyz | 1] lhsT columns
    HW_ = H * W                      # 288
    NS = N // 512                    # 512-wide m slices (6)
    SC_SCALE = T / float(np.sqrt(np.float32(Dh)))
    GB = BETA / (2.0 * BAND_WIDTH * BAND_WIDTH)   # 0.25: weight on d2

    nc = bacc.Bacc("TRN2", target_bir_lowering=False, debug=False,
                   num_devices=R)

    din = {}
    def dram_in(name, shape, dtype):
        din[name] = nc.dram_tensor(name, list(shape), dtype, kind="ExternalInput")
        return din[name]

    xT_d = dram_in("xT", [D, N], bf16)            # x^T replicated
    xlocTb_d = dram_in("xlocTb", [D, L], bf16)    # x[rows]^T
    xlocTf_d = dram_in("xlocTf", [D, L], f32)
    dymT_d = dram_in("dymT", [N, L], bf16)        # masked delta_y, transposed
    xyzg0_d = dram_in("xyzg0", [R, 6 * L], f32)   # initial xyz, allgather layout
    xyzlocT0_d = dram_in("xyzlocT0", [3, L], f32)
    wq_d = dram_in("wq", [D, D], bf16)
    wk_d = dram_in("wk", [D, D], bf16)
    wv_d = dram_in("wv", [D, D], bf16)
    wo_d = dram_in("wo", [D, D], bf16)
    negi_d = dram_in("negi", [128, 128], mybir.dt.bfloat16)  # -2*I
    selg_d = dram_in("selg", [128, 32], f32)      # s-row gather pattern
    selh_d = dram_in("selh", [8, 512], f32)       # recip scatter pattern
    selp_d = dram_in("selp", [128, 4], f32)       # head-sum pattern
    i3_d = dram_in("i3", [3, 3], f32)

    outT_d = nc.dram_tensor("outT", [D, L], f32, kind="ExternalOutput")
    xyzout_d = nc.dram_tensor("xyzout", [L, 3], f32, kind="ExternalOutput")

    with tile.TileContext(nc) as tc:
        with (
            tc.tile_pool(name="const", bufs=1) as cp,
            tc.tile_pool(name="dram", bufs=1, space="DRAM") as dp,
            tc.tile_pool(name="xts", bufs=2) as xts_p,
            tc.tile_pool(name="kts", bufs=2) as kts_p,
            tc.tile_pool(name="dymz", bufs=3) as dymz_p,
            tc.tile_pool(name="ez", bufs=2) as ez_p,
            tc.tile_pool(name="u4", bufs=3) as u4_p,
            tc.tile_pool(name="xyzts", bufs=2) as xyzts_p,
            tc.tile_pool(name="fxyz", bufs=2) as fxyz_p,
            tc.tile_pool(name="scb", bufs=2) as scb_p,
        ):
            # ---- persistent SBUF tiles ----
            ME = cp.tile([128, C * LH], bf16)        # masked exp(scores)
            vw = cp.tile([128, C * HW_], bf16)       # [v_h | xyz | 1] blocks
            qT = cp.tile([128, 2 * L], bf16)
            xlocTb = cp.tile([128, 2 * L], bf16)
            xlocTf = cp.tile([128, 2 * L], f32)
            wq_t = cp.tile([128, 2 * D], bf16)
            wk_t = cp.tile([128, 2 * D], bf16)
            wv_t = cp.tile([128, 2 * D], bf16)
            wo_t = cp.tile([128, 2 * D], bf16)
            natf32 = cp.tile([128, 3 * C], f32)
            sqt = cp.tile([128, 3 * C], f32)
            negnorm = cp.tile([128, C], f32)
            natloc = cp.tile([128, 3 * CL], f32)
            xyzlocT = cp.tile([4, L], f32)
            rsP = cp.tile([128, 4 * L], f32)         # per-bank s/xyz row copies
            scaledb = cp.tile([128, L], f32)         # per-bank normalized rows
            recipS8 = cp.tile([8, L], f32)
            xyzlocT67 = cp.tile([67, L], bf16)       # hi/lo split of xyzlocT
            negi = cp.tile([128, 128], bf16)
            selg = cp.tile([128, 32], f32)
            selh = cp.tile([8, 512], f32)
            selp = cp.tile([128, 4], f32)
            i3sb = cp.tile([3, 3], f32)

            agin = dp.tile([1, 6 * L], f32)
            agout = dp.tile([R, 6 * L], f32)

            dma = nc.sync.dma_start
            dma(out=xlocTb[:, :].rearrange("p (k l) -> p k l", l=L),
                in_=xlocTb_d[:, :].rearrange("(k p) l -> p k l", p=128))
            dma(out=xlocTf[:, :].rearrange("p (k l) -> p k l", l=L),
                in_=xlocTf_d[:, :].rearrange("(k p) l -> p k l", p=128))
            for wt, wd in ((wq_t, wq_d), (wk_t, wk_d), (wv_t, wv_d), (wo_t, wo_d)):
                dma(out=wt[:, :].rearrange("p (k d) -> p k d", d=D),
                    in_=wd[:, :].rearrange("(k p) d -> p k d", p=128))
            dma(out=negi[:, :], in_=negi_d[:, :])
            dma(out=selg[:, :], in_=selg_d[:, :])
            dma(out=selh[:, :], in_=selh_d[:, :])
            dma(out=selp[:, :], in_=selp_d[:, :])
            dma(out=i3sb[:, :], in_=i3_d[:, :])
            dma(out=xyzlocT[0:3, :], in_=xyzlocT0_d[:, :])
            # rsP/scaledb rows outside the written ones are read (x0 weight)
            # by the selector matmuls -> must be finite.
            nc.vector.memset(rsP[:, :], 0.0)
            nc.vector.memset(scaledb[:, :], 0.0)

            # ---- phase A PSUM pools (closed before the iteration pools open
            # so the 8 banks can be re-used) ----
            from contextlib import ExitStack
            pa_stack = ExitStack()
            psS_p = pa_stack.enter_context(
                tc.tile_pool(name="psS", bufs=2, space="PSUM"))

            # ---- qT = Wq^T @ xloc^T ----
            psq = psS_p.tile([128, 2048], f32, tag="pss", name="psq")
            for oc in range(2):
                for kc in range(2):
                    nc.tensor.matmul(psq[:, 512 * oc: 512 * oc + L],
                                     lhsT=wq_t[:, D * kc + 128 * oc: D * kc + 128 * oc + 128],
                                     rhs=xlocTb[:, L * kc: L * kc + L],
                                     start=(kc == 0), stop=(kc == 1))
            nc.vector.tensor_copy(
                qT[:, :].rearrange("p (o l) -> p o l", l=L),
                psq[:, 0:1024].rearrange("p (o x) -> p o x", x=512)[:, :, 0:L])

            # ---- phase A ----
            for s in range(NS):
                xts = xts_p.tile([128, 1024], bf16, tag="xts")
                dma(out=xts[:, :].rearrange("p (k m) -> p k m", m=512),
                    in_=xT_d[:, 512 * s: 512 * s + 512]
                        .rearrange("(k p) m -> p k m", p=128))
                kts = kts_p.tile([128, 1024], bf16, tag="kts")
                psk = psS_p.tile([128, 2048], f32, tag="pss", name="psk")
                for oc in range(2):
                    for kc in range(2):
                        nc.tensor.matmul(psk[:, 512 * oc: 512 * oc + 512],
                                         lhsT=wk_t[:, D * kc + 128 * oc: D * kc + 128 * oc + 128],
                                         rhs=xts[:, 512 * kc: 512 * kc + 512],
                                         start=(kc == 0), stop=(kc == 1))
                nc.vector.tensor_copy(kts[:, :], psk[:, 0:1024])
                psv = psS_p.tile([128, 2048], f32, tag="pss", name="psv")
                for cc in range(4):
                    for kc in range(2):
                        nc.tensor.matmul(psv[:, 512 * cc: 512 * cc + D],
                                         lhsT=xts[:, 512 * kc + 128 * cc: 512 * kc + 128 * cc + 128],
                                         rhs=wv_t[:, D * kc: D * kc + D],
                                         start=(kc == 0), stop=(kc == 1))
                nc.vector.tensor_copy(
                    vw[:, HW_ * 4 * s: HW_ * 4 * s + 4 * HW_]
                        .rearrange("p (c h w) -> p c h w", h=H, w=W)[:, :, :, 0:Dh],
                    psv[:, :].rearrange("p (c x) -> p c x", x=512)
                        [:, :, 0:D].rearrange("p c (h j) -> p c h j", j=Dh))
                nc.vector.memset(
                    vw[:, HW_ * 4 * s: HW_ * 4 * s + 4 * HW_]
                        .rearrange("p (c h w) -> p c h w", h=H, w=W)
                        [:, :, :, Dh + 3:Dh + 4],
                    1.0)
                for cc in range(4):
                    c = 4 * s + cc
                    for g in range(2):
                        psS = psS_p.tile([128, 2048], f32, tag="pss", name="psS")
                        for i in range(4):
                            nc.tensor.matmul(
                                psS[:, 512 * i: 512 * i + L],
                                lhsT=kts[32 * i: 32 * i + 32,
                                         512 * g + 128 * cc: 512 * g + 128 * cc + 128],
                                rhs=qT[32 * i: 32 * i + 32, L * g: L * g + L],
                                start=True, stop=True,
                                tile_position=(32 * i, 0))
                        nc.scalar.activation(
                            ME[:, LH * c + 4 * L * g: LH * c + 4 * L * g + 4 * L]
                                .rearrange("p (h q) -> p h q", q=L),
                            psS[:, :].rearrange("p (b q) -> p b q", q=512)[:, :, 0:L],
                            Exp, scale=SC_SCALE)

            pa_stack.close()
            psG_p = tc.tile_pool(name="psG", bufs=2, space="PSUM")
            psP_p = tc.tile_pool(name="psP", bufs=4, space="PSUM")
            psM_p = tc.tile_pool(name="psM", bufs=2, space="PSUM")
            it_stack = ExitStack()
            psG_p = it_stack.enter_context(psG_p)
            psP_p = it_stack.enter_context(psP_p)
            psM_p = it_stack.enter_context(psM_p)

            # ---- geometry prep (runs before each iteration's chunks) ----
            def prep_geometry(src):
                for r in range(R):
                    dma(out=natf32[:, 3 * CL * r: 3 * CL * (r + 1)]
                            .rearrange("p (t d) -> p t d", d=3),
                        in_=src[r:r + 1, 0:3 * L]
                            .rearrange("o (t p d) -> p (o t) d", p=128, d=3))
                # hi/lo bf16 split of the local xyz^T for the G matmuls.
                # row pairing with the lhsT side: 0-2 hi*hi, 32-34 hi_m*lo_q,
                # 64-66 lo_m*hi_q (lo*lo is negligible).
                nc.vector.memset(xyzlocT67[0:64, :], 0.0)
                nc.vector.tensor_copy(xyzlocT67[0:3, :], xyzlocT[0:3, :])
                nc.vector.tensor_sub(xyzlocT67[32:35, :], xyzlocT[0:3, :],
                                     xyzlocT67[0:3, :])
                nc.vector.tensor_copy(xyzlocT67[64:67, :], xyzlocT67[0:3, :])
                nc.vector.tensor_mul(sqt[:, :], natf32[:, :], natf32[:, :])
                nc.vector.tensor_reduce(
                    negnorm[:, :].rearrange("p (c o) -> p c o", o=1),
                    sqt[:, :].rearrange("p (c d) -> p c d", d=3),
                    axis=AX, op=mybir.AluOpType.add)
                nc.vector.tensor_scalar_mul(negnorm[:, :], negnorm[:, :], -GB)
                nc.vector.tensor_copy(
                    vw[:, :].rearrange("p (c h w) -> p c h w", h=H, w=W)
                        [:, :, :, Dh:Dh + 3],
                    natf32[:, :].rearrange("p (c o d) -> p c o d", o=1, d=3)
                        .broadcast_to([128, C, H, 3]))

            prep_geometry(xyzg0_d)

            # ---- iterations ----
            for it in range(MAX_ITER):
                last = (it == MAX_ITER - 1)

                psP = [psP_p.tile([128, 512], f32, tag="psp",
                                  name=f"psP_{it}_{bb}") for bb in range(4)]

                xyzts = None
                for c in range(C):
                    rb, t = divmod(c, CL)
                    if t == 0:
                        fxyz = fxyz_p.tile([3, L], f32, tag="fxyz")
                        src = xyzg0_d if it == 0 else agout
                        dma(out=fxyz[:, :],
                            in_=src[rb:rb + 1, 3 * L: 6 * L]
                                .rearrange("o (d q) -> (o d) q", q=L))
                        xyzts = xyzts_p.tile([67, L], bf16, tag="xyzts")
                        nc.vector.memset(xyzts[0:64, :], 0.0)
                        nc.vector.tensor_copy(xyzts[0:3, :], fxyz[:, :])
                        nc.vector.tensor_copy(xyzts[32:35, :], xyzts[0:3, :])
                        nc.vector.tensor_sub(xyzts[64:67, :], fxyz[:, :],
                                             xyzts[0:3, :])
                    zt = dymz_p.tile([128, L], bf16, tag="dymz")
                    dma(out=zt[:, :], in_=dymT_d[128 * c: 128 * c + 128, :])
                    psG = psG_p.tile([128, 512], f32, tag="psg")
                    nc.tensor.matmul(psG[:, 0:L],
                                     lhsT=xyzts[0:67, 128 * t: 128 * t + 128],
                                     rhs=xyzlocT67[0:67, :],
                                     start=True, stop=False)
                    nc.tensor.matmul(psG[:, 0:L],
                                     lhsT=negi[:, :], rhs=zt[:, :],
                                     start=False, stop=True)
                    et = ez_p.tile([128, L], bf16, tag="ez")
                    nc.scalar.activation(et[:, :], psG[:, 0:L], Exp,
                                         bias=negnorm[:, c:c + 1],
                                         scale=2.0 * GB)
                    et_b = et[:, :].rearrange("p (o q) -> p o q", o=1)                         .broadcast_to([128, 2, L])
                    for pair in range(4):
                        u4 = u4_p.tile([128, 2 * L], bf16, tag="u4")
                        nc.vector.tensor_mul(
                            u4[:, :].rearrange("p (o q) -> p o q", q=L),
                            ME[:, LH * c + 2 * L * pair: LH * c + 2 * L * pair + 2 * L]
                                .rearrange("p (o q) -> p o q", q=L),
                            et_b)
                        for i in range(2):
                            h = 2 * pair + i
                            b, hi = h % 4, h // 4
                            nc.tensor.matmul(
                                psP[b][64 * hi: 64 * hi + W, 0:L],
                                lhsT=vw[:, HW_ * c + W * h: HW_ * c + W * h + W],
                                rhs=u4[:, L * i: L * i + L],
                                start=(c == 0), stop=(c == C - 1),
                                skip_group_check=True)

                # ---- boundary: normalize heads, update xyz ----
                # psP rows per 64-block: feat@0-31, xyz@32-34, s@35.
                for b in range(4):
                    for hi in range(2):
                        p0 = 64 * hi + 32
                        nc.vector.tensor_copy(rsP[p0:p0 + 4, L * b: L * b + L],
                                              psP[b][p0:p0 + 4, 0:L])
                # gather the 8 s rows -> [8, L], one batched reciprocal,
                # then scatter 1/s (and 1/(H s)) back per bank.
                psS8 = psM_p.tile([128, 512], f32, tag="psm", name="psS8")
                for b in range(4):
                    nc.tensor.matmul(psS8[0:8, 0:L],
                                     lhsT=selg[:, 8 * b: 8 * b + 8],
                                     rhs=rsP[:, L * b: L * b + L],
                                     start=(b == 0), stop=(b == 3))
                nc.vector.reciprocal(recipS8[0:8, :], psS8[0:8, 0:L])
                psX = psM_p.tile([128, 512], f32, tag="psm", name="psX")
                for b in range(4):
                    psR = psM_p.tile([128, 512], f32, tag="psm", name="psR")
                    nc.tensor.matmul(psR[:, 0:L],
                                     lhsT=selh[:, 128 * b: 128 * b + 128],
                                     rhs=recipS8[0:8, :],
                                     start=True, stop=True)
                    rsb = scb_p.tile([128, L], f32, tag="scb")
                    nc.vector.tensor_copy(rsb[:, :], psR[:, 0:L])
                    # only the matmul-written PSUM rows are touched (elsewhere
                    # PSUM may hold non-finite garbage); scaledb stays 0 there.
                    for hi in range(2):
                        p0 = 64 * hi
                        nc.vector.tensor_mul(scaledb[p0:p0 + W, :],
                                             psP[b][p0:p0 + W, 0:L],
                                             rsb[p0:p0 + W, :])
                    nc.tensor.matmul(psX[0:4, 0:L],
                                     lhsT=selp[:, :],
                                     rhs=scaledb[:, :],
                                     start=(b == 0), stop=(b == 3))
                    if last:
                        for hi in range(2):
                            nc.vector.tensor_copy(
                                qT[32 * b: 32 * b + 32, L * hi: L * hi + L],
                                scaledb[64 * hi: 64 * hi + 32, :])
                nc.vector.tensor_copy(xyzlocT[0:4, :], psX[0:4, 0:L])

                for t in range(CL):
                    psN = psM_p.tile([128, 512], f32, tag="psm")
                    nc.tensor.matmul(psN[:, 0:3],
                                     lhsT=xyzlocT[0:3, 128 * t: 128 * t + 128],
                                     rhs=i3sb[:, :], start=True, stop=True)
                    nc.vector.tensor_copy(natloc[:, 3 * t: 3 * t + 3], psN[:, 0:3])

                if last:
                    dma(out=xyzout_d[:, :].rearrange("(t p) d -> p t d", p=128),
                        in_=natloc[:, :].rearrange("p (t d) -> p t d", d=3))
                    for oc in range(2):
                        psO = psM_p.tile([128, 512], f32, tag="psm")
                        for kc in range(2):
                            nc.tensor.matmul(
                                psO[:, 0:L],
                                lhsT=wo_t[:, D * kc + 128 * oc: D * kc + 128 * oc + 128],
                                rhs=qT[:, L * kc: L * kc + L],
                                start=(kc == 0), stop=(kc == 1))
                        nc.vector.tensor_add(xlocTf[:, L * oc: L * oc + L],
                                             psO[:, 0:L],
                                             xlocTf[:, L * oc: L * oc + L])
                        dma(out=outT_d[128 * oc: 128 * oc + 128, :],
                            in_=xlocTf[:, L * oc: L * oc + L])
                else:
                    dma(out=agin[0:1, 0:3 * L].rearrange("o (t p d) -> p (o t) d",
                                                         p=128, d=3),
                        in_=natloc[:, :].rearrange("p (t d) -> p t d", d=3))
                    dma(out=agin[0:1, 3 * L: 6 * L]
                            .rearrange("o (d q) -> (o d) q", q=L),
                        in_=xyzlocT[0:3, :])
                    nc.gpsimd.collective_compute(
                        "AllGather", mybir.AluOpType.bypass,
                        replica_groups=[list(range(R))],
                        ins=[agin[:, :].opt()], outs=[agout[:, :].opt()])
                    prep_geometry(agout)

            it_stack.close()

    nc.compile()
    return nc


def _get_nc(N, D, R):
    key = (N, D, R)
    if key not in _cache:
        _cache[key] = _build(N, D, R)
    return _cache[key]


def _make_consts(H):
    """Selector matmul patterns for head normalization (see boundary code)."""
    selg = np.zeros((128, 32), np.float32)    # gather s rows -> [8, L]
    selh = np.zeros((8, 512), np.float32)     # scatter 1/s back per bank
    selp = np.zeros((128, 4), np.float32)     # sum xyz rows across heads
    for b in range(4):
        for hi in range(2):
            h = 4 * hi + b
            selg[64 * hi + 35, 8 * b + h] = 1.0
            for j in range(32):
                selh[h, 128 * b + 64 * hi + j] = 1.0
            for c in range(3):
                selh[h, 128 * b + 64 * hi + 32 + c] = 1.0 / H
    for hi in range(2):
        for c in range(3):
            selp[64 * hi + 32 + c, c] = 1.0
    return selg, selh, selp


def _host_prep(x, xyz, delta_y, distance_mask, big_inter_mask,
               Wq, bq, Wk, bk, Wv, bv, Wo, bo, R):
    import concourse.mybir as mybir
    bf = mybir.dt.np(mybir.dt.bfloat16)
    N, D = x.shape
    L = N // R

    for b in (bq, bk, bv, bo):
        assert np.max(np.abs(np.asarray(b))) == 0.0, "nonzero biases unsupported"

    x = np.asarray(x, np.float32)
    xyz = np.asarray(xyz, np.float32)
    delta_y = np.asarray(delta_y, np.float32)
    valid = (np.asarray(distance_mask, bool) & np.asarray(big_inter_mask, bool))
    np.fill_diagonal(valid, True)
    dym = np.where(valid, delta_y, DYM_MASKED).astype(np.float32)

    xT_bf = np.ascontiguousarray(x.T).astype(bf)
    wq_bf = np.ascontiguousarray(np.asarray(Wq, np.float32)).astype(bf)
    wk_bf = np.ascontiguousarray(np.asarray(Wk, np.float32)).astype(bf)
    wv_bf = np.ascontiguousarray(np.asarray(Wv, np.float32)).astype(bf)
    wo_bf = np.ascontiguousarray(np.asarray(Wo, np.float32)).astype(bf)

    xyzg0 = np.zeros((R, 6 * L), np.float32)
    for r in range(R):
        blk = xyz[L * r: L * r + L]
        xyzg0[r, 0:3 * L] = blk.reshape(-1)
        xyzg0[r, 3 * L:6 * L] = np.ascontiguousarray(blk.T).reshape(-1)

    selg, selh, selp = _make_consts(HEADS)
    i3 = np.eye(3, dtype=np.float32)
    negi = (-2.0 * np.eye(128, dtype=np.float32)).astype(bf)

    in_maps = []
    for r in range(R):
        rows = slice(L * r, L * (r + 1))
        in_maps.append(dict(
            xT=xT_bf,
            xlocTb=np.ascontiguousarray(x[rows].T).astype(bf),
            xlocTf=np.ascontiguousarray(x[rows].T),
            dymT=np.ascontiguousarray(dym[rows].T).astype(bf),
            xyzg0=xyzg0,
            xyzlocT0=np.ascontiguousarray(xyz[rows].T),
            wq=wq_bf, wk=wk_bf, wv=wv_bf, wo=wo_bf,
            selg=selg, selh=selh, selp=selp, i3=i3, negi=negi,
        ))
    return in_maps


def run(inputs, R=8, trace=False):
    from concourse.bass_utils import run_bass_kernel_spmd
    N, D = inputs["x"].shape
    nc = _get_nc(N, D, R)
    in_maps = _host_prep(R=R, **inputs)
    res = run_bass_kernel_spmd(nc, in_maps, list(range(R)), trace=trace)
    L = N // R
    out = np.zeros((N, D), np.float32)
    xyz_out = np.zeros((N, 3), np.float32)
    for r in range(R):
        out[L * r: L * r + L] = np.asarray(res.results[r]["outT"], np.float32).T
        xyz_out[L * r: L * r + L] = np.asarray(res.results[r]["xyzout"], np.float32)
    return (xyz_out, out), res


def kernel(**inputs):
    (xyz_out, out), _ = run(inputs, R=8, trace=False)
    return xyz_out, out
